# revision 34
# baseline (speedup 1.0000x reference)
"""Trainium2 Bass kernel for nn_AttentionBlock (GroupNorm -> MHA -> proj + residual).

Contract: kernel(**inputs) takes the FULL unsharded inputs (as produced by
setup_inputs) and returns the FULL output [8, 512, 32, 32] float32.

Sharding: pure data-parallel over batch B=8 across the 8 NeuronCores; each core
processes one batch element end-to-end (no collectives needed).

Per-core layout / algorithm (B=1, C=512, N=H*W=1024, heads=8, head_dim=64):
  - All matmuls bf16 (fp8 DoubleRow was measured to trigger a hardware
    power throttle to half clock, netting ~0 gain while slowing neighbors).
  - GroupNorm(32 groups) in fp32; bn_stats/bn_aggr per 128-channel tile,
    batched group-combine + broadcast via tiny PE matmuls, DVE-only rsqrt.
    All elementwise on DVE (gpsimd tensor ops measured at ~14.7us per
    [128,1024] tile -- unusable).
  - Attention in "S^T" layout: S^T[m,n] = sum_c k[c,m] q[c,n], K=64 bf16.
    exp on ScalarE -> bf16 E tiles ([128,2,512], 64 ACTIVATEs).
    AV with lhsT = [ones(64) | v(64)] per head: PSUM rows 0:64 hold the
    softmax denominator broadcast (partition offset 0 so
    reciprocal_approx_fast can read it directly), rows 64:128 hold O.
  - Blocks are half-major: b=0..7 -> (pr=b%4, half=b//4); O accumulates per
    block in one PSUM tile [128, 2(hi), 512]. Epilogue = 1 reciprocal +
    2 tensor_tensor mults straight out of PSUM (no copies).
  - proj per (r, half) split kc 0:2 / 2:4; first half fuses x + pb via
    scalar_tensor_tensor, second half adds and streams the output DMA
    inside the attention stream (only the last half=1 quarter is tail).
  - v-bias folded into pb on host (pb_eff = proj_b + proj_w @ b_v); q scale
    folded into wq/bq.
  - Static drip schedule interleaves qkv/vt/proj matmuls into the attention
    stream keyed on exp-tile index; AV lags exp by LAG units (software
    pipeline) so the PE never waits on ScalarE.
"""

import numpy as np
import ml_dtypes

import concourse.bass as bass
import concourse.tile as tile
from concourse import bacc, mybir
from concourse.bass_utils import run_bass_kernel_spmd

FP32 = mybir.dt.float32
BF16 = mybir.dt.bfloat16
AF = mybir.ActivationFunctionType
OP = mybir.AluOpType

P = 128      # SBUF partitions
C = 512      # channels
NT = 1024    # spatial tokens (32*32)
CT = C // P  # channel tiles = 4
MT = NT // P # key tiles = 8
NH = 8       # heads
HD = 64      # head dim
NCORES = 8
GSZ = 16     # channels per group (512/32)

LAG = 6  # AV units behind exp


def _emit(tc: "tile.TileContext", io: dict):
    nc = tc.nc
    from collections import deque
    import contextlib
    ctx = contextlib.ExitStack()
    with ctx:
        pers = ctx.enter_context(tc.tile_pool(name="pers", bufs=1))
        sm = ctx.enter_context(tc.tile_pool(name="small", bufs=1))

        x, wq, wk, wv, pw = io["x"], io["wq"], io["wk"], io["wv"], io["pw"]
        out = io["out"]

        # ---------------- input DMAs ----------------
        x_r = x.rearrange("(r p) n -> p r n", p=P)
        x_sb = pers.tile([P, CT, NT], FP32, tag="x")
        nc.sync.dma_start(x_sb[:, 0, :], x_r[:, 0, :])
        nc.gpsimd.dma_start(x_sb[:, 1, :], x_r[:, 1, :])
        nc.scalar.dma_start(x_sb[:, 2, :], x_r[:, 2, :])
        nc.sync.dma_start(x_sb[:, 3, :], x_r[:, 3, :])
        # small tensors on the scalar queue (idle during the head)
        amat_sb = pers.tile([P, NH], FP32, tag="amat")
        nc.scalar.dma_start(amat_sb, io["amat"])
        imat_sb = pers.tile([NH, P], FP32, tag="imat")
        nc.scalar.dma_start(imat_sb, io["imat"])
        gg_sb = pers.tile([P, CT], FP32, tag="gg")
        nc.scalar.dma_start(gg_sb, io["gg"].rearrange("(r p) -> p r", p=P))
        gb_sb = pers.tile([P, CT], FP32, tag="gb")
        nc.scalar.dma_start(gb_sb, io["gb"].rearrange("(r p) -> p r", p=P))
        bq_sb = pers.tile([P, CT], FP32, tag="bq")
        nc.scalar.dma_start(bq_sb, io["bq"].rearrange("(r p) -> p r", p=P))
        bk_sb = pers.tile([P, CT], FP32, tag="bk")
        nc.scalar.dma_start(bk_sb, io["bk"].rearrange("(r p) -> p r", p=P))
        pb_sb = pers.tile([P, CT], FP32, tag="pb")
        nc.scalar.dma_start(pb_sb, io["pb"].rearrange("(r p) -> p r", p=P))
        # weights: host-packed to final SBUF layout [p, kc, oc]
        wq_sb = pers.tile([P, CT, C], BF16, tag="wq")
        nc.sync.dma_start(wq_sb, wq)
        wk_sb = pers.tile([P, CT, C], BF16, tag="wk")
        nc.gpsimd.dma_start(wk_sb, wk)
        wv_sb = pers.tile([P, CT, C], BF16, tag="wv")
        nc.gpsimd.dma_start(wv_sb, wv)
        pw_sb = pers.tile([P, CT, C], BF16, tag="pw")
        nc.scalar.dma_start(pw_sb, pw)

        # preload the exp activation table while DMAs are in flight
        warm_sb = pers.tile([1, 1], FP32, tag="actwarm")
        nc.vector.memset(warm_sb, 0.0)
        nc.scalar.activation(warm_sb, warm_sb, AF.Exp)

        # persistent SBUF
        h_sb = pers.tile([P, CT, NT], BF16, tag="h")
        q_sb = pers.tile([P, CT, NT], BF16, tag="q")
        k_sb = pers.tile([P, CT, NT], BF16, tag="k")
        # vT per head block: cols 0:64 = ones (denominator), 64:128 = v
        vT_sb = pers.tile([P, MT, NH * P], BF16, tag="vT")
        O_sb = pers.tile([P, CT, NT], BF16, tag="O")
        P1x_sb = pers.tile([P, CT, NT], FP32, tag="p1x")

        nc.gpsimd.memset(
            vT_sb.rearrange("p t (h c) -> p t h c", c=P)[:, :, :, 0:HD], 1.0)

        # ---------------- GroupNorm ----------------
        with nc.named_scope("gn"), \
             tc.tile_pool(name="gnps", bufs=1, space="PSUM") as gnps, \
             tc.tile_pool(name="mrps", bufs=1, space="PSUM") as mrps:
            st2_all = sm.tile([P, CT, 2], FP32, tag="st2_all")
            mv_all = sm.tile([P, CT, 2], FP32, tag="mv_all")
            for r in range(CT):
                st = sm.tile([P, 2, 6], FP32, tag=f"bnstats{r}")
                nc.vector.bn_stats(st[:, 0, :], x_sb[:, r, 0:512])
                nc.vector.bn_stats(st[:, 1, :], x_sb[:, r, 512:1024])
                nc.vector.bn_aggr(mv_all[:, r, :], st)
            nc.vector.tensor_copy(st2_all[:, :, 0:1], mv_all[:, :, 0:1])
            nc.vector.tensor_tensor(st2_all[:, :, 1:2], mv_all[:, :, 0:1],
                                    mv_all[:, :, 0:1], OP.mult)
            nc.vector.tensor_tensor(st2_all[:, :, 1:2], st2_all[:, :, 1:2],
                                    mv_all[:, :, 1:2], OP.add)
            G_ps = gnps.tile([NH, CT, 2], FP32, tag="gps")
            nc.tensor.matmul(G_ps, amat_sb,
                             st2_all.rearrange("p r k -> p (r k)"),
                             start=True, stop=True)
            st_all = sm.tile([NH, CT, 2], FP32, tag="st_all")
            nc.vector.tensor_copy(st_all, G_ps)
            var_all = sm.tile([NH, CT], FP32, tag="var_all")
            nc.vector.tensor_tensor(var_all[:, :, None], st_all[:, :, 0:1],
                                    st_all[:, :, 0:1], OP.mult)
            nc.vector.tensor_tensor(var_all[:, :, None], st_all[:, :, 1:2],
                                    var_all[:, :, None], OP.subtract)
            # rstd = rsqrt(var + eps): 1/v seed + 2 Newton steps on DVE
            nc.vector.tensor_scalar(var_all, var_all, 1e-5, None, OP.add)
            y = sm.tile([NH, CT], FP32, tag="rsqrt_y")
            nc.vector.reciprocal_approx_fast(y, var_all)
            t = sm.tile([NH, CT], FP32, tag="rsqrt_t")
            for it in range(2):
                nc.vector.tensor_tensor(t, y, y, OP.mult)
                nc.vector.tensor_tensor(t, t, var_all, OP.mult)
                nc.vector.tensor_scalar(t, t, -0.5, 1.5, OP.mult, OP.add)
                if it < 1:
                    nc.vector.tensor_tensor(y, y, t, OP.mult)
                else:
                    nc.vector.tensor_tensor(st_all[:, :, 1:2], y[:, :, None],
                                            t[:, :, None], OP.mult)
            MR_ps = mrps.tile([P, CT, 2], FP32, tag="mrps")
            nc.tensor.matmul(MR_ps, imat_sb,
                             st_all.rearrange("p r k -> p (r k)"),
                             start=True, stop=True)
            mr = sm.tile([P, CT, 2], FP32, tag="mr")
            nc.vector.tensor_copy(mr, MR_ps)
            a_all = sm.tile([P, CT, 1], FP32, tag="gn_a")
            nc.vector.tensor_tensor(a_all, mr[:, :, 1:2], gg_sb[:, :, None],
                                    OP.mult)
            b_all = sm.tile([P, CT, 1], FP32, tag="gn_b")
            nc.vector.tensor_tensor(b_all, mr[:, :, 0:1], a_all, OP.mult)
            nc.vector.tensor_tensor(b_all, gb_sb[:, :, None], b_all,
                                    OP.subtract)
            for r in range(CT):
                nc.vector.tensor_scalar(h_sb[:, r, :], x_sb[:, r, :],
                                        a_all[:, r, :], b_all[:, r, :],
                                        OP.mult, OP.add)

        # ------------- qkv + attention -------------
        with nc.named_scope("qkv_attn"), \
             tc.tile_pool(name="spool", bufs=1, space="PSUM") as spool, \
             tc.tile_pool(name="opool", bufs=1, space="PSUM") as opool, \
             tc.tile_pool(name="bgps", bufs=1, space="PSUM") as bgps, \
             tc.tile_pool(name="epool", bufs=6) as epool, \
             tc.tile_pool(name="rpool", bufs=2) as rpool, \
             tc.tile_pool(name="outp", bufs=4) as outp:

            out_r = out.rearrange("(r p) n -> p r n", p=P)

            bg_i = [0]

            def bg_tile(name):
                bg_i[0] ^= 1
                return bgps.tile([P, 512], FP32, tag=f"bg{bg_i[0]}",
                                 name=name)

            def qk_task(dst, w_sb, b_sb, r, half, on_scalar=False):
                ps = bg_tile(f"qk_{r}_{half}_{w_sb.name}")
                for kc in range(CT):
                    nc.tensor.matmul(
                        ps, w_sb[:, kc, P * r:P * r + P],
                        h_sb[:, kc, 512 * half:512 * half + 512],
                        start=(kc == 0), stop=(kc == CT - 1))
                dst_ap = dst[:, r, 512 * half:512 * half + 512]
                if on_scalar:
                    nc.scalar.add(dst_ap, ps, b_sb[:, r:r + 1])
                else:
                    nc.vector.tensor_scalar(dst_ap, ps, b_sb[:, r:r + 1],
                                            None, OP.add)

            def vt_task(t):
                ps = bg_tile(f"vt{t}")
                for kc in range(CT):
                    nc.tensor.matmul(ps, h_sb[:, kc, P * t:P * t + P],
                                     wv_sb[:, kc, :],
                                     start=(kc == 0), stop=(kc == CT - 1))
                nc.vector.tensor_copy(
                    vT_sb[:, t, :].rearrange("p (h c) -> p h c",
                                             c=P)[:, :, HD:P],
                    ps.rearrange("p (h c) -> p h c", c=HD))

            def proj01_task(r, half):
                hs = 512 * half
                ps = bg_tile(f"pjA_{r}_{half}")
                for kc in range(2):
                    nc.tensor.matmul(ps, pw_sb[:, kc, P * r:P * r + P],
                                     O_sb[:, kc, hs:hs + 512],
                                     start=(kc == 0), stop=(kc == 1))
                # P1x = (ps + pb) + x in one fused DVE op
                nc.vector.scalar_tensor_tensor(
                    P1x_sb[:, r, hs:hs + 512], ps, pb_sb[:, r:r + 1],
                    x_sb[:, r, hs:hs + 512], OP.add, OP.add)

            def proj23_task(r, half):
                hs = 512 * half
                ps = bg_tile(f"pjB_{r}_{half}")
                for kc in range(2, 4):
                    nc.tensor.matmul(ps, pw_sb[:, kc, P * r:P * r + P],
                                     O_sb[:, kc, hs:hs + 512],
                                     start=(kc == 2), stop=(kc == 3))
                o_st = outp.tile([P, 512], FP32, tag="ost",
                                 name=f"ost{r}_{half}")
                nc.vector.tensor_tensor(o_st, ps,
                                        P1x_sb[:, r, hs:hs + 512], OP.add)
                if half == 0:
                    eng = nc.sync if r % 2 == 0 else nc.gpsimd
                    eng.dma_start(out_r[:, r, hs:hs + 512], o_st)
                else:
                    engs = (nc.sync, nc.gpsimd, nc.scalar)
                    engs[(2 * r) % 3].dma_start(
                        out_r[:, r, hs:hs + 256], o_st[:, 0:256])
                    engs[(2 * r + 1) % 3].dma_start(
                        out_r[:, r, hs + 256:hs + 512], o_st[:, 256:512])

            # upfront: deps of block 0 (pr0, half0); copies ride ScalarE
            qk_task(k_sb, wk_sb, bk_sb, 0, 0, on_scalar=True)
            qk_task(k_sb, wk_sb, bk_sb, 0, 1, on_scalar=True)
            qk_task(q_sb, wq_sb, bq_sb, 0, 0, on_scalar=True)

            # drip schedule: exp-tile index (0..63) -> tasks. blocks are
            # half-major: b = 0..7 -> (pr = b % 4, half = b // 4); epilogue
            # of block b is emitted around tile 8b + 8 + LAG/2.
            drip = {
                0: [(vt_task, (0,)), (vt_task, (1,))],
                1: [(vt_task, (2,)), (vt_task, (3,))],
                2: [(vt_task, (4,)), (vt_task, (5,))],
                3: [(vt_task, (6,)), (vt_task, (7,))],
                4: [(qk_task, (k_sb, wk_sb, bk_sb, 1, 0))],
                5: [(qk_task, (k_sb, wk_sb, bk_sb, 1, 1))],
                6: [(qk_task, (q_sb, wq_sb, bq_sb, 1, 0))],
                9: [(qk_task, (k_sb, wk_sb, bk_sb, 2, 0))],
                11: [(qk_task, (k_sb, wk_sb, bk_sb, 2, 1))],
                13: [(qk_task, (q_sb, wq_sb, bq_sb, 2, 0))],
                17: [(qk_task, (k_sb, wk_sb, bk_sb, 3, 0))],
                19: [(qk_task, (k_sb, wk_sb, bk_sb, 3, 1))],
                21: [(qk_task, (q_sb, wq_sb, bq_sb, 3, 0))],
                25: [(qk_task, (q_sb, wq_sb, bq_sb, 0, 1))],
                20: [(proj01_task, (0, 0))],
                22: [(proj01_task, (1, 0))],
                24: [(proj01_task, (2, 0))],
                26: [(proj01_task, (3, 0))],
                33: [(qk_task, (q_sb, wq_sb, bq_sb, 1, 1))],
                37: [(proj23_task, (0, 0))],
                39: [(proj23_task, (1, 0))],
                41: [(proj23_task, (2, 0))],
                43: [(proj23_task, (3, 0))],
                45: [(qk_task, (q_sb, wq_sb, bq_sb, 2, 1))],
                49: [(qk_task, (q_sb, wq_sb, bq_sb, 3, 1))],
                53: [(proj01_task, (0, 1))],
                55: [(proj01_task, (1, 1))],
                57: [(proj01_task, (2, 1))],
                59: [(proj01_task, (3, 1))],
            }

            O_cur = [None]

            def emit_av(b, hi, t, E_t, j):
                pr, half = b % 4, b // 4
                if hi == 0 and t == 0:
                    O_cur[0] = opool.tile([P, 2, 512], FP32, tag="o",
                                          name=f"o{b}")
                h = 2 * pr + hi
                nc.tensor.matmul(
                    O_cur[0][:, hi, :],
                    vT_sb[:, t, P * h:P * h + P],
                    E_t[:, j, :],
                    start=(t == 0), stop=(t == MT - 1))

            def emit_epilogue(b):
                pr, half = b % 4, b // 4
                hs = 512 * half
                O_pair = O_cur[0]
                Rh = rpool.tile([HD, 2, 512], FP32, tag="rh", name=f"rh{b}")
                # D is broadcast on PSUM rows 0:64 (partition offset 0, as
                # the fast reciprocal requires)
                nc.vector.reciprocal_approx_fast(Rh, O_pair[0:HD, :, :])
                for hi in range(2):
                    nc.vector.tensor_tensor(
                        O_sb[HD * hi:HD * hi + HD, pr, hs:hs + 512],
                        O_pair[HD:P, hi, :], Rh[:, hi, :], OP.mult)

            pend = deque()

            def flush_unit():
                b, hi, t, E_t, j = pend.popleft()
                emit_av(b, hi, t, E_t, j)
                if hi == 1 and t == MT - 1:
                    emit_epilogue(b)

            s_i = [0]
            for ti in range(64):
                b, rem = ti // 8, ti % 8
                pr, half = b % 4, b // 4
                hi, u = rem // 4, rem % 4
                s_i[0] ^= 1
                S_t = spool.tile([P, 2, 512], FP32, tag=f"s{s_i[0]}",
                                 name=f"st{ti}")
                for j in range(2):
                    t = 2 * u + j
                    nc.tensor.matmul(
                        S_t[:, j, :],
                        k_sb[HD * hi:HD * hi + HD, pr, P * t:P * t + P],
                        q_sb[HD * hi:HD * hi + HD, pr,
                             512 * half:512 * half + 512],
                        start=True, stop=True)
                E_t = epool.tile([P, 2, 512], BF16, tag="e", name=f"et{ti}")
                nc.scalar.activation(E_t, S_t, AF.Exp)
                for j in range(2):
                    pend.append((b, hi, 2 * u + j, E_t, j))
                lag = LAG if ti < 58 else 3
                while len(pend) > lag:
                    flush_unit()
                for fn, args in drip.pop(ti, ()):
                    fn(*args)
            while pend:
                flush_unit()
            assert not drip, f"undripped: {list(drip)}"

            # ---------------- tail: proj kc 2:4 for half 1 ----------------
            with nc.named_scope("proj_tail"):
                for r in range(CT):
                    proj23_task(r, 1)


_CACHE: dict = {}


def _build():
    if "nc" in _CACHE:
        return _CACHE["nc"]
    nc = bacc.Bacc("TRN2", target_bir_lowering=False, debug=False,
                   num_devices=NCORES)
    io = {
        "x": nc.dram_tensor("x", [C, NT], FP32, kind="ExternalInput").ap(),
        "wq": nc.dram_tensor("wq", [P, CT, C], BF16, kind="ExternalInput").ap(),
        "wk": nc.dram_tensor("wk", [P, CT, C], BF16, kind="ExternalInput").ap(),
        "wv": nc.dram_tensor("wv", [P, CT, C], BF16, kind="ExternalInput").ap(),
        "pw": nc.dram_tensor("pw", [P, CT, C], BF16, kind="ExternalInput").ap(),
        "bq": nc.dram_tensor("bq", [C], FP32, kind="ExternalInput").ap(),
        "bk": nc.dram_tensor("bk", [C], FP32, kind="ExternalInput").ap(),
        "pb": nc.dram_tensor("pb", [C], FP32, kind="ExternalInput").ap(),
        "gg": nc.dram_tensor("gg", [C], FP32, kind="ExternalInput").ap(),
        "gb": nc.dram_tensor("gb", [C], FP32, kind="ExternalInput").ap(),
        "amat": nc.dram_tensor("amat", [P, NH], FP32, kind="ExternalInput").ap(),
        "imat": nc.dram_tensor("imat", [NH, P], FP32, kind="ExternalInput").ap(),
        "out": nc.dram_tensor("out", [C, NT], FP32, kind="ExternalOutput").ap(),
    }
    with tile.TileContext(nc) as tc:
        _emit(tc, io)
    nc.compile()
    _CACHE["nc"] = nc
    return nc


def _host_prep(inputs):
    x = np.ascontiguousarray(np.asarray(inputs["x"], dtype=np.float32))
    qkv_w = np.asarray(inputs["qkv_w"], dtype=np.float32)
    qkv_b = np.asarray(inputs["qkv_b"], dtype=np.float32)
    proj_w = np.asarray(inputs["proj_w"], dtype=np.float32)
    proj_b = np.asarray(inputs["proj_b"], dtype=np.float32)
    gn_scale = np.asarray(inputs["gn_scale"], dtype=np.float32)
    gn_bias = np.asarray(inputs["gn_bias"], dtype=np.float32)

    s = np.float32(1.0 / np.sqrt(HD))
    bf = ml_dtypes.bfloat16

    def pack_qk(w):
        # [p, kc, oc] = w[oc, 128*kc + p]
        return np.ascontiguousarray(
            w.reshape(C, CT, P).transpose(2, 1, 0)).astype(bf)

    shared = {
        "wq": pack_qk(qkv_w[0:C] * s),
        "wk": pack_qk(qkv_w[C:2 * C]),
        "wv": pack_qk(qkv_w[2 * C:3 * C]),
        "pw": pack_qk(proj_w),
        "bq": (qkv_b[0:C] * s).astype(np.float32),
        "bk": qkv_b[C:2 * C].astype(np.float32),
        # v bias and proj bias folded: proj(o + b_v) = proj(o) + W_p b_v
        "pb": (proj_b + proj_w @ qkv_b[2 * C:3 * C]).astype(np.float32),
        "gg": gn_scale,
        "gb": gn_bias,
        "amat": (np.kron(np.eye(NH, dtype=np.float32),
                         np.ones((GSZ, 1), np.float32)) / GSZ),
        "imat": np.ascontiguousarray(np.kron(np.eye(NH, dtype=np.float32),
                                             np.ones((1, GSZ), np.float32))),
    }
    B = x.shape[0]
    in_maps = []
    for b in range(B):
        m = dict(shared)
        m["x"] = np.ascontiguousarray(x[b].reshape(C, NT))
        in_maps.append(m)
    return in_maps


def run(inputs, trace=False):
    nc = _build()
    in_maps = _host_prep(inputs)
    res = run_bass_kernel_spmd(nc, in_maps, list(range(NCORES)), trace=trace)
    out = np.stack([res.results[i]["out"] for i in range(NCORES)], axis=0)
    return out.reshape(len(in_maps), C, 32, 32), res


def kernel(**inputs) -> np.ndarray:
    out, _ = run(inputs, trace=False)
    return out.astype(np.float32)


# revision 35
# speedup vs baseline: 1.0177x; 1.0177x over previous
"""Trainium2 Bass kernel for nn_AttentionBlock (GroupNorm -> MHA -> proj + residual).

Contract: kernel(**inputs) takes the FULL unsharded inputs (as produced by
setup_inputs) and returns the FULL output [8, 512, 32, 32] float32.

Sharding: pure data-parallel over batch B=8 across the 8 NeuronCores; each core
processes one batch element end-to-end (no collectives needed).

Per-core layout / algorithm (B=1, C=512, N=H*W=1024, heads=8, head_dim=64):
  - All matmuls bf16 (fp8 DoubleRow was measured to trigger a hardware
    power throttle to half clock, netting ~0 gain while slowing neighbors).
  - GroupNorm(32 groups) in fp32; bn_stats/bn_aggr per 128-channel tile,
    batched group-combine + broadcast via tiny PE matmuls, DVE-only rsqrt.
    All elementwise on DVE (gpsimd tensor ops measured at ~14.7us per
    [128,1024] tile -- unusable).
  - Attention in "S^T" layout: S^T[m,n] = sum_c k[c,m] q[c,n], K=64 bf16.
    exp on ScalarE -> bf16 E tiles ([128,2,512], 64 ACTIVATEs).
    AV with lhsT = [ones(64) | v(64)] per head: PSUM rows 0:64 hold the
    softmax denominator broadcast (partition offset 0 so
    reciprocal_approx_fast can read it directly), rows 64:128 hold O.
  - Blocks are half-major: b=0..7 -> (pr=b%4, half=b//4); O accumulates per
    block in one PSUM tile [128, 2(hi), 512]. Epilogue = 1 reciprocal +
    2 tensor_tensor mults straight out of PSUM (no copies).
  - proj per (r, half) split kc 0:2 / 2:4; first half fuses x + pb via
    scalar_tensor_tensor, second half adds and streams the output DMA
    inside the attention stream (only the last half=1 quarter is tail).
  - v-bias folded into pb on host (pb_eff = proj_b + proj_w @ b_v); q scale
    folded into wq/bq.
  - Static drip schedule interleaves qkv/vt/proj matmuls into the attention
    stream keyed on exp-tile index; AV lags exp by LAG units (software
    pipeline) so the PE never waits on ScalarE.
"""

import numpy as np
import ml_dtypes

import concourse.bass as bass
import concourse.tile as tile
from concourse import bacc, mybir
from concourse.bass_utils import run_bass_kernel_spmd

FP32 = mybir.dt.float32
BF16 = mybir.dt.bfloat16
AF = mybir.ActivationFunctionType
OP = mybir.AluOpType

P = 128      # SBUF partitions
C = 512      # channels
NT = 1024    # spatial tokens (32*32)
CT = C // P  # channel tiles = 4
MT = NT // P # key tiles = 8
NH = 8       # heads
HD = 64      # head dim
NCORES = 8
GSZ = 16     # channels per group (512/32)

LAG = 6  # AV units behind exp


def _emit(tc: "tile.TileContext", io: dict):
    nc = tc.nc
    from collections import deque
    import contextlib
    ctx = contextlib.ExitStack()
    with ctx:
        pers = ctx.enter_context(tc.tile_pool(name="pers", bufs=1))
        sm = ctx.enter_context(tc.tile_pool(name="small", bufs=1))

        x, wq, wk, wv, pw = io["x"], io["wq"], io["wk"], io["wv"], io["pw"]
        out = io["out"]

        # ---------------- input DMAs ----------------
        x_r = x.rearrange("(r p) n -> p r n", p=P)
        x_sb = pers.tile([P, CT, NT], FP32, tag="x")
        nc.sync.dma_start(x_sb[:, 0, :], x_r[:, 0, :])
        nc.gpsimd.dma_start(x_sb[:, 1, :], x_r[:, 1, :])
        nc.scalar.dma_start(x_sb[:, 2, :], x_r[:, 2, :])
        nc.sync.dma_start(x_sb[:, 3, :], x_r[:, 3, :])
        # small tensors on the scalar queue (idle during the head)
        amat_sb = pers.tile([P, NH], FP32, tag="amat")
        nc.scalar.dma_start(amat_sb, io["amat"])
        imat_sb = pers.tile([NH, P], FP32, tag="imat")
        nc.scalar.dma_start(imat_sb, io["imat"])
        gg_sb = pers.tile([P, CT], FP32, tag="gg")
        nc.scalar.dma_start(gg_sb, io["gg"].rearrange("(r p) -> p r", p=P))
        gb_sb = pers.tile([P, CT], FP32, tag="gb")
        nc.scalar.dma_start(gb_sb, io["gb"].rearrange("(r p) -> p r", p=P))
        bq_sb = pers.tile([P, CT], FP32, tag="bq")
        nc.scalar.dma_start(bq_sb, io["bq"].rearrange("(r p) -> p r", p=P))
        bk_sb = pers.tile([P, CT], FP32, tag="bk")
        nc.scalar.dma_start(bk_sb, io["bk"].rearrange("(r p) -> p r", p=P))
        pb_sb = pers.tile([P, CT], FP32, tag="pb")
        nc.scalar.dma_start(pb_sb, io["pb"].rearrange("(r p) -> p r", p=P))
        # weights: host-packed to final SBUF layout [p, kc, oc]
        wq_sb = pers.tile([P, CT, C], BF16, tag="wq")
        nc.sync.dma_start(wq_sb, wq)
        wk_sb = pers.tile([P, CT, C], BF16, tag="wk")
        nc.gpsimd.dma_start(wk_sb, wk)
        wv_sb = pers.tile([P, CT, C], BF16, tag="wv")
        nc.gpsimd.dma_start(wv_sb, wv)
        pw_sb = pers.tile([P, CT, C], BF16, tag="pw")
        nc.scalar.dma_start(pw_sb, pw)

        # preload the exp activation table while DMAs are in flight
        warm_sb = pers.tile([1, 1], FP32, tag="actwarm")
        nc.vector.memset(warm_sb, 0.0)
        nc.scalar.activation(warm_sb, warm_sb, AF.Exp)

        # persistent SBUF
        h_sb = pers.tile([P, CT, NT], BF16, tag="h")
        q_sb = pers.tile([P, CT, NT], BF16, tag="q")
        k_sb = pers.tile([P, CT, NT], BF16, tag="k")
        # vT per head block: cols 0:64 = ones (denominator), 64:128 = v
        vT_sb = pers.tile([P, MT, NH * P], BF16, tag="vT")
        O_sb = pers.tile([P, CT, NT], BF16, tag="O")
        P1x_sb = pers.tile([P, CT, NT], FP32, tag="p1x")

        nc.gpsimd.memset(
            vT_sb.rearrange("p t (h c) -> p t h c", c=P)[:, :, :, 0:HD], 1.0)

        # ---------------- GroupNorm ----------------
        with nc.named_scope("gn"), \
             tc.tile_pool(name="gnps", bufs=1, space="PSUM") as gnps, \
             tc.tile_pool(name="mrps", bufs=1, space="PSUM") as mrps:
            st2_all = sm.tile([P, CT, 2], FP32, tag="st2_all")
            mv_all = sm.tile([P, CT, 2], FP32, tag="mv_all")
            for r in range(CT):
                st = sm.tile([P, 2, 6], FP32, tag=f"bnstats{r}")
                nc.vector.bn_stats(st[:, 0, :], x_sb[:, r, 0:512])
                nc.vector.bn_stats(st[:, 1, :], x_sb[:, r, 512:1024])
                nc.vector.bn_aggr(mv_all[:, r, :], st)
            nc.vector.tensor_copy(st2_all[:, :, 0:1], mv_all[:, :, 0:1])
            nc.vector.tensor_tensor(st2_all[:, :, 1:2], mv_all[:, :, 0:1],
                                    mv_all[:, :, 0:1], OP.mult)
            nc.vector.tensor_tensor(st2_all[:, :, 1:2], st2_all[:, :, 1:2],
                                    mv_all[:, :, 1:2], OP.add)
            G_ps = gnps.tile([NH, CT, 2], FP32, tag="gps")
            nc.tensor.matmul(G_ps, amat_sb,
                             st2_all.rearrange("p r k -> p (r k)"),
                             start=True, stop=True)
            st_all = sm.tile([NH, CT, 2], FP32, tag="st_all")
            nc.vector.tensor_copy(st_all, G_ps)
            var_all = sm.tile([NH, CT], FP32, tag="var_all")
            nc.vector.tensor_tensor(var_all[:, :, None], st_all[:, :, 0:1],
                                    st_all[:, :, 0:1], OP.mult)
            nc.vector.tensor_tensor(var_all[:, :, None], st_all[:, :, 1:2],
                                    var_all[:, :, None], OP.subtract)
            # rstd = rsqrt(var + eps): 1/v seed + 2 Newton steps on DVE
            nc.vector.tensor_scalar(var_all, var_all, 1e-5, None, OP.add)
            y = sm.tile([NH, CT], FP32, tag="rsqrt_y")
            nc.vector.reciprocal_approx_fast(y, var_all)
            t = sm.tile([NH, CT], FP32, tag="rsqrt_t")
            for it in range(2):
                nc.vector.tensor_tensor(t, y, y, OP.mult)
                nc.vector.tensor_tensor(t, t, var_all, OP.mult)
                nc.vector.tensor_scalar(t, t, -0.5, 1.5, OP.mult, OP.add)
                if it < 1:
                    nc.vector.tensor_tensor(y, y, t, OP.mult)
                else:
                    nc.vector.tensor_tensor(st_all[:, :, 1:2], y[:, :, None],
                                            t[:, :, None], OP.mult)
            MR_ps = mrps.tile([P, CT, 2], FP32, tag="mrps")
            nc.tensor.matmul(MR_ps, imat_sb,
                             st_all.rearrange("p r k -> p (r k)"),
                             start=True, stop=True)
            mr = sm.tile([P, CT, 2], FP32, tag="mr")
            nc.vector.tensor_copy(mr, MR_ps)
            a_all = sm.tile([P, CT, 1], FP32, tag="gn_a")
            nc.vector.tensor_tensor(a_all, mr[:, :, 1:2], gg_sb[:, :, None],
                                    OP.mult)
            b_all = sm.tile([P, CT, 1], FP32, tag="gn_b")
            nc.vector.tensor_tensor(b_all, mr[:, :, 0:1], a_all, OP.mult)
            nc.vector.tensor_tensor(b_all, gb_sb[:, :, None], b_all,
                                    OP.subtract)
            for r in range(CT):
                nc.vector.tensor_scalar(h_sb[:, r, :], x_sb[:, r, :],
                                        a_all[:, r, :], b_all[:, r, :],
                                        OP.mult, OP.add)

        # ------------- qkv + attention -------------
        with nc.named_scope("qkv_attn"), \
             tc.tile_pool(name="spool", bufs=1, space="PSUM") as spool, \
             tc.tile_pool(name="opool", bufs=1, space="PSUM") as opool, \
             tc.tile_pool(name="bgps", bufs=1, space="PSUM") as bgps, \
             tc.tile_pool(name="epool", bufs=6) as epool, \
             tc.tile_pool(name="rpool", bufs=2) as rpool, \
             tc.tile_pool(name="outp", bufs=4) as outp:

            out_r = out.rearrange("(r p) n -> p r n", p=P)

            bg_i = [0]

            def bg_tile(name):
                bg_i[0] ^= 1
                return bgps.tile([P, 512], FP32, tag=f"bg{bg_i[0]}",
                                 name=name)

            def qk_task(dst, w_sb, b_sb, r, half, on_scalar=False):
                ps = bg_tile(f"qk_{r}_{half}_{w_sb.name}")
                for kc in range(CT):
                    nc.tensor.matmul(
                        ps, w_sb[:, kc, P * r:P * r + P],
                        h_sb[:, kc, 512 * half:512 * half + 512],
                        start=(kc == 0), stop=(kc == CT - 1))
                dst_ap = dst[:, r, 512 * half:512 * half + 512]
                if on_scalar:
                    nc.scalar.add(dst_ap, ps, b_sb[:, r:r + 1])
                else:
                    nc.vector.tensor_scalar(dst_ap, ps, b_sb[:, r:r + 1],
                                            None, OP.add)

            def vt_task(t):
                ps = bg_tile(f"vt{t}")
                for kc in range(CT):
                    nc.tensor.matmul(ps, h_sb[:, kc, P * t:P * t + P],
                                     wv_sb[:, kc, :],
                                     start=(kc == 0), stop=(kc == CT - 1))
                nc.vector.tensor_copy(
                    vT_sb[:, t, :].rearrange("p (h c) -> p h c",
                                             c=P)[:, :, HD:P],
                    ps.rearrange("p (h c) -> p h c", c=HD))

            def proj01_task(r, half):
                hs = 512 * half
                ps = bg_tile(f"pjA_{r}_{half}")
                for kc in range(2):
                    nc.tensor.matmul(ps, pw_sb[:, kc, P * r:P * r + P],
                                     O_sb[:, kc, hs:hs + 512],
                                     start=(kc == 0), stop=(kc == 1))
                # P1x = (ps + pb) + x in one fused DVE op
                nc.vector.scalar_tensor_tensor(
                    P1x_sb[:, r, hs:hs + 512], ps, pb_sb[:, r:r + 1],
                    x_sb[:, r, hs:hs + 512], OP.add, OP.add)

            def proj23_task(r, half):
                hs = 512 * half
                ps = bg_tile(f"pjB_{r}_{half}")
                for kc in range(2, 4):
                    nc.tensor.matmul(ps, pw_sb[:, kc, P * r:P * r + P],
                                     O_sb[:, kc, hs:hs + 512],
                                     start=(kc == 2), stop=(kc == 3))
                o_st = outp.tile([P, 512], FP32, tag="ost",
                                 name=f"ost{r}_{half}")
                nc.vector.tensor_tensor(o_st, ps,
                                        P1x_sb[:, r, hs:hs + 512], OP.add)
                eng = nc.sync if (r + half) % 2 == 0 else nc.gpsimd
                eng.dma_start(out_r[:, r, hs:hs + 512], o_st)

            # upfront: deps of block 0 (pr0, half0); copies ride ScalarE
            qk_task(k_sb, wk_sb, bk_sb, 0, 0, on_scalar=True)
            qk_task(k_sb, wk_sb, bk_sb, 0, 1, on_scalar=True)
            qk_task(q_sb, wq_sb, bq_sb, 0, 0, on_scalar=True)

            # drip schedule: exp-tile index (0..63) -> tasks. blocks are
            # half-major: b = 0..7 -> (pr = b % 4, half = b // 4); epilogue
            # of block b is emitted around tile 8b + 8 + LAG/2.
            drip = {
                0: [(vt_task, (0,)), (vt_task, (1,))],
                1: [(vt_task, (2,)), (vt_task, (3,))],
                2: [(vt_task, (4,)), (vt_task, (5,))],
                3: [(vt_task, (6,)), (vt_task, (7,))],
                4: [(qk_task, (k_sb, wk_sb, bk_sb, 1, 0))],
                5: [(qk_task, (k_sb, wk_sb, bk_sb, 1, 1))],
                6: [(qk_task, (q_sb, wq_sb, bq_sb, 1, 0))],
                9: [(qk_task, (k_sb, wk_sb, bk_sb, 2, 0))],
                11: [(qk_task, (k_sb, wk_sb, bk_sb, 2, 1))],
                13: [(qk_task, (q_sb, wq_sb, bq_sb, 2, 0))],
                17: [(qk_task, (k_sb, wk_sb, bk_sb, 3, 0))],
                19: [(qk_task, (k_sb, wk_sb, bk_sb, 3, 1))],
                21: [(qk_task, (q_sb, wq_sb, bq_sb, 3, 0))],
                25: [(qk_task, (q_sb, wq_sb, bq_sb, 0, 1))],
                20: [(proj01_task, (0, 0))],
                22: [(proj01_task, (1, 0))],
                24: [(proj01_task, (2, 0))],
                26: [(proj01_task, (3, 0))],
                33: [(qk_task, (q_sb, wq_sb, bq_sb, 1, 1))],
                37: [(proj23_task, (0, 0))],
                39: [(proj23_task, (1, 0))],
                41: [(proj23_task, (2, 0))],
                43: [(proj23_task, (3, 0))],
                45: [(qk_task, (q_sb, wq_sb, bq_sb, 2, 1))],
                49: [(qk_task, (q_sb, wq_sb, bq_sb, 3, 1))],
                53: [(proj01_task, (0, 1))],
                55: [(proj01_task, (1, 1))],
                57: [(proj01_task, (2, 1))],
                59: [(proj01_task, (3, 1))],
            }

            O_cur = [None]

            def emit_av(b, hi, t, E_t, j):
                pr, half = b % 4, b // 4
                if hi == 0 and t == 0:
                    O_cur[0] = opool.tile([P, 2, 512], FP32, tag="o",
                                          name=f"o{b}")
                h = 2 * pr + hi
                nc.tensor.matmul(
                    O_cur[0][:, hi, :],
                    vT_sb[:, t, P * h:P * h + P],
                    E_t[:, j, :],
                    start=(t == 0), stop=(t == MT - 1))

            def emit_epilogue(b):
                pr, half = b % 4, b // 4
                hs = 512 * half
                O_pair = O_cur[0]
                Rh = rpool.tile([HD, 2, 512], FP32, tag="rh", name=f"rh{b}")
                # D is broadcast on PSUM rows 0:64 (partition offset 0, as
                # the fast reciprocal requires)
                nc.vector.reciprocal_approx_fast(Rh, O_pair[0:HD, :, :])
                for hi in range(2):
                    nc.vector.tensor_tensor(
                        O_sb[HD * hi:HD * hi + HD, pr, hs:hs + 512],
                        O_pair[HD:P, hi, :], Rh[:, hi, :], OP.mult)

            pend = deque()

            def flush_unit():
                b, hi, t, E_t, j = pend.popleft()
                emit_av(b, hi, t, E_t, j)
                if hi == 1 and t == MT - 1:
                    emit_epilogue(b)

            s_i = [0]
            for ti in range(64):
                b, rem = ti // 8, ti % 8
                pr, half = b % 4, b // 4
                hi, u = rem // 4, rem % 4
                s_i[0] ^= 1
                S_t = spool.tile([P, 2, 512], FP32, tag=f"s{s_i[0]}",
                                 name=f"st{ti}")
                for j in range(2):
                    t = 2 * u + j
                    nc.tensor.matmul(
                        S_t[:, j, :],
                        k_sb[HD * hi:HD * hi + HD, pr, P * t:P * t + P],
                        q_sb[HD * hi:HD * hi + HD, pr,
                             512 * half:512 * half + 512],
                        start=True, stop=True)
                E_t = epool.tile([P, 2, 512], BF16, tag="e", name=f"et{ti}")
                nc.scalar.activation(E_t, S_t, AF.Exp)
                for j in range(2):
                    pend.append((b, hi, 2 * u + j, E_t, j))
                while len(pend) > LAG:
                    flush_unit()
                for fn, args in drip.pop(ti, ()):
                    fn(*args)
            while pend:
                flush_unit()
            assert not drip, f"undripped: {list(drip)}"

            # ---------------- tail: proj kc 2:4 for half 1 ----------------
            with nc.named_scope("proj_tail"):
                for r in range(CT):
                    proj23_task(r, 1)


_CACHE: dict = {}


def _build():
    if "nc" in _CACHE:
        return _CACHE["nc"]
    nc = bacc.Bacc("TRN2", target_bir_lowering=False, debug=False,
                   num_devices=NCORES)
    io = {
        "x": nc.dram_tensor("x", [C, NT], FP32, kind="ExternalInput").ap(),
        "wq": nc.dram_tensor("wq", [P, CT, C], BF16, kind="ExternalInput").ap(),
        "wk": nc.dram_tensor("wk", [P, CT, C], BF16, kind="ExternalInput").ap(),
        "wv": nc.dram_tensor("wv", [P, CT, C], BF16, kind="ExternalInput").ap(),
        "pw": nc.dram_tensor("pw", [P, CT, C], BF16, kind="ExternalInput").ap(),
        "bq": nc.dram_tensor("bq", [C], FP32, kind="ExternalInput").ap(),
        "bk": nc.dram_tensor("bk", [C], FP32, kind="ExternalInput").ap(),
        "pb": nc.dram_tensor("pb", [C], FP32, kind="ExternalInput").ap(),
        "gg": nc.dram_tensor("gg", [C], FP32, kind="ExternalInput").ap(),
        "gb": nc.dram_tensor("gb", [C], FP32, kind="ExternalInput").ap(),
        "amat": nc.dram_tensor("amat", [P, NH], FP32, kind="ExternalInput").ap(),
        "imat": nc.dram_tensor("imat", [NH, P], FP32, kind="ExternalInput").ap(),
        "out": nc.dram_tensor("out", [C, NT], FP32, kind="ExternalOutput").ap(),
    }
    with tile.TileContext(nc) as tc:
        _emit(tc, io)
    nc.compile()
    _CACHE["nc"] = nc
    return nc


def _host_prep(inputs):
    x = np.ascontiguousarray(np.asarray(inputs["x"], dtype=np.float32))
    qkv_w = np.asarray(inputs["qkv_w"], dtype=np.float32)
    qkv_b = np.asarray(inputs["qkv_b"], dtype=np.float32)
    proj_w = np.asarray(inputs["proj_w"], dtype=np.float32)
    proj_b = np.asarray(inputs["proj_b"], dtype=np.float32)
    gn_scale = np.asarray(inputs["gn_scale"], dtype=np.float32)
    gn_bias = np.asarray(inputs["gn_bias"], dtype=np.float32)

    s = np.float32(1.0 / np.sqrt(HD))
    bf = ml_dtypes.bfloat16

    def pack_qk(w):
        # [p, kc, oc] = w[oc, 128*kc + p]
        return np.ascontiguousarray(
            w.reshape(C, CT, P).transpose(2, 1, 0)).astype(bf)

    shared = {
        "wq": pack_qk(qkv_w[0:C] * s),
        "wk": pack_qk(qkv_w[C:2 * C]),
        "wv": pack_qk(qkv_w[2 * C:3 * C]),
        "pw": pack_qk(proj_w),
        "bq": (qkv_b[0:C] * s).astype(np.float32),
        "bk": qkv_b[C:2 * C].astype(np.float32),
        # v bias and proj bias folded: proj(o + b_v) = proj(o) + W_p b_v
        "pb": (proj_b + proj_w @ qkv_b[2 * C:3 * C]).astype(np.float32),
        "gg": gn_scale,
        "gb": gn_bias,
        "amat": (np.kron(np.eye(NH, dtype=np.float32),
                         np.ones((GSZ, 1), np.float32)) / GSZ),
        "imat": np.ascontiguousarray(np.kron(np.eye(NH, dtype=np.float32),
                                             np.ones((1, GSZ), np.float32))),
    }
    B = x.shape[0]
    in_maps = []
    for b in range(B):
        m = dict(shared)
        m["x"] = np.ascontiguousarray(x[b].reshape(C, NT))
        in_maps.append(m)
    return in_maps


def run(inputs, trace=False):
    nc = _build()
    in_maps = _host_prep(inputs)
    res = run_bass_kernel_spmd(nc, in_maps, list(range(NCORES)), trace=trace)
    out = np.stack([res.results[i]["out"] for i in range(NCORES)], axis=0)
    return out.reshape(len(in_maps), C, 32, 32), res


def kernel(**inputs) -> np.ndarray:
    out, _ = run(inputs, trace=False)
    return out.astype(np.float32)
